# revision 8
# baseline (speedup 1.0000x reference)
"""Multi-head causal attention with RoPE on 8 Trainium2 NeuronCores.

Reference computation (B=2, T=2048, C=1024, H=16, Dh=64, fp32):
    qkv = x @ w_qkv + b_qkv ; split q,k,v ; RoPE(q), RoPE(k)
    attn = softmax_causal(q k^T / sqrt(Dh)) @ v ; out = attn @ w_proj + b_proj

Sharding: core c = b*4 + g handles batch b and head group g (heads 4g..4g+3).
Data-parallel over batch, tensor-parallel over heads (w_qkv column-split,
w_proj row-split).  Each core emits a partial [T, C] projection output; the
host sums the 4 per-batch partials and adds b_proj.

Per-core kernel (all matmuls in fp32r: full PE rate, ~1e-3 relerr):
  - x^T is pre-transposed on the host, so QKV runs with weights stationary
    producing Q^T/K^T directly in [head_dim, T] layout; V in [T, head_dim].
  - biases are folded into the PSUM accumulation as rank-1 (K=1) matmuls.
  - RoPE: rope(q) = q*cos + shift32(q*sin_perm), the partition shift done
    with a constant 128x128 0/1 permutation matmul.
  - attention per (head, 512-wide q-span): S^T tile = K_j Q^T (scores
    transposed, so softmax reduction direction = PE contraction), exp on
    ACT with fused 1/sqrt(Dh) scale (no max subtraction: scores are ~N(0,1),
    fp32 exp cannot overflow), causal handled by narrowing each k-tile's
    q-range plus one triangular mask on the diagonal 128x128 block.
  - V is augmented with a ones column so the PV matmul also produces the
    softmax denominator (row 64); normalization happens after PV via
    reciprocal + rank-1 broadcast matmul.
  - projection: per head-pair stationary attn^T tiles vs w_proj rows.
"""

import numpy as np

import concourse.bacc as bacc
import concourse.mybir as mybir
from concourse.tile import TileContext
from concourse.bass_utils import run_bass_kernel_spmd

F32 = mybir.dt.float32
F32R = mybir.dt.float32r

B, T, C = 2, 2048, 1024
H, DH = 16, 64
GH = 4  # heads per core
N_CORES = 8
NCHUNK = C // 128  # 8 contraction chunks
NT = T // 128  # 16 token tiles
NSPAN = T // 512  # 4 query spans
QK_COLS = 2 * GH * DH  # 512 = q cols (256) + k cols (256)
VA = GH * (DH + 1)  # 260 = v cols augmented with ones column per head


def _build():
    nc = bacc.Bacc("TRN2", target_bir_lowering=False, debug=False, num_devices=N_CORES)

    xT = nc.dram_tensor("xT", [C, T], F32R, kind="ExternalInput")
    wqk = nc.dram_tensor("wqk", [C, QK_COLS], F32R, kind="ExternalInput")
    wv = nc.dram_tensor("wv", [C, VA], F32R, kind="ExternalInput")
    bqk_d = nc.dram_tensor("bqk", [1, QK_COLS], F32R, kind="ExternalInput")
    bv_d = nc.dram_tensor("bv", [1, VA], F32R, kind="ExternalInput")
    cos_d = nc.dram_tensor("cosT", [128, T], F32, kind="ExternalInput")
    sinp_d = nc.dram_tensor("sinTp", [128, T], F32, kind="ExternalInput")
    perm_d = nc.dram_tensor("perm", [128, 128], F32R, kind="ExternalInput")
    tri_d = nc.dram_tensor("tri", [128, 128], F32, kind="ExternalInput")
    wproj_d = nc.dram_tensor("wproj", [2, 128, C], F32R, kind="ExternalInput")
    out_d = nc.dram_tensor("out", [T, C], F32, kind="ExternalOutput")

    with TileContext(nc) as tc:
        with tc.tile_pool(name="persist", bufs=1) as pers:
            # Constants / small inputs (live whole kernel)
            ones_f = pers.tile([128, 512], F32, tag="ones_f")
            nc.vector.memset(ones_f, 1.0)
            ones = pers.tile([128, 512], F32R, tag="ones")
            nc.vector.tensor_copy(ones, ones_f)
            cos_sb = pers.tile([128, T], F32, tag="cos")
            nc.sync.dma_start(out=cos_sb, in_=cos_d[:, :])
            sinp_sb = pers.tile([128, T], F32, tag="sinp")
            nc.sync.dma_start(out=sinp_sb, in_=sinp_d[:, :])
            perm_sb = pers.tile([128, 128], F32R, tag="perm")
            nc.sync.dma_start(out=perm_sb, in_=perm_d[:, :])
            tri_sb = pers.tile([128, 128], F32, tag="tri")
            nc.sync.dma_start(out=tri_sb, in_=tri_d[:, :])
            bqk_sb = pers.tile([1, QK_COLS], F32R, tag="bqk")
            nc.sync.dma_start(out=bqk_sb, in_=bqk_d[:, :])
            bv_sb = pers.tile([1, VA], F32R, tag="bv")
            nc.sync.dma_start(out=bv_sb, in_=bv_d[:, :])

            # Outputs of phase 1 (live into phase 2/3)
            qkt = []  # 4 tiles [128, T]: Q heads(0,1), Q(2,3), K(0,1), K(2,3)
            for i in range(4):
                t = pers.tile([128, T], F32R, tag="qkt", bufs=4, name=f"qkt{i}")
                qkt.append(t)
            vaug = []  # 16 tiles [128, VA], k-tile-major natural layout V
            for j in range(NT):
                t = pers.tile([128, VA], F32R, tag="vaug", bufs=NT, name=f"vaug{j}")
                vaug.append(t)
            attn = []  # 2 tiles [128, T]: normalized attn^T for head pairs
            for p in range(2):
                t = pers.tile([128, T], F32R, tag="attn", bufs=2, name=f"attn{p}")
                attn.append(t)

            # ---------------- Phase 1: QKV projection + RoPE ----------------
            with (
                tc.tile_pool(name="p1", bufs=1) as p1,
                tc.tile_pool(name="p1ps", bufs=1, space="PSUM") as p1ps,
            ):
                xt = []
                for kc in range(NCHUNK):
                    t = p1.tile([128, T], F32R, tag="xt", bufs=NCHUNK, name=f"xt{kc}")
                    nc.sync.dma_start(out=t, in_=xT[128 * kc : 128 * (kc + 1), :])
                    xt.append(t)
                wqk_t = []
                for kc in range(NCHUNK):
                    t = p1.tile(
                        [128, QK_COLS], F32R, tag="wqk", bufs=NCHUNK, name=f"wqk{kc}"
                    )
                    nc.sync.dma_start(out=t, in_=wqk[128 * kc : 128 * (kc + 1), :])
                    wqk_t.append(t)
                wv_t = []
                for kc in range(NCHUNK):
                    t = p1.tile([128, VA], F32R, tag="wv", bufs=NCHUNK, name=f"wv{kc}")
                    nc.sync.dma_start(out=t, in_=wv[128 * kc : 128 * (kc + 1), :])
                    wv_t.append(t)

                # V natural layout: for each token tile, [128 tok, VA cols]
                for it in range(NT):
                    pv = p1ps.tile([128, VA], F32, tag="psv", bufs=2, name="psv")
                    ts = slice(128 * it, 128 * (it + 1))
                    for kc in range(NCHUNK):
                        nc.tensor.matmul(
                            pv,
                            xt[kc][:, ts],
                            wv_t[kc],
                            start=(kc == 0),
                            stop=False,
                        )
                    # bias (includes the ones column): pv[t, c] += bv[c]
                    nc.tensor.matmul(
                        pv, ones[0:1, 0:128], bv_sb, start=False, stop=True
                    )
                    nc.scalar.copy(vaug[it], pv)

                # Q^T / K^T col-tiles with fused bias + RoPE
                for ct in range(4):
                    cs = slice(128 * ct, 128 * (ct + 1))
                    for sp in range(NSPAN):
                        ss = slice(512 * sp, 512 * (sp + 1))
                        pq = p1ps.tile([128, 512], F32, tag="psqk", bufs=2, name="psqk")
                        for kc in range(NCHUNK):
                            nc.tensor.matmul(
                                pq,
                                wqk_t[kc][:, cs],
                                xt[kc][:, ss],
                                start=(kc == 0),
                                stop=False,
                            )
                        nc.tensor.matmul(
                            pq, bqk_sb[0:1, cs], ones[0:1, :], start=False, stop=True
                        )
                        # rope: qkt = pq*cos + perm @ (pq*sin_perm)
                        t2 = p1.tile([128, 512], F32R, tag="t2", bufs=3, name="t2")
                        nc.vector.tensor_mul(t2, pq, sinp_sb[:, ss])
                        pp = p1ps.tile(
                            [128, 512], F32, tag="psperm", bufs=2, name="psperm"
                        )
                        nc.tensor.matmul(pp, perm_sb, t2, start=True, stop=True)
                        nc.vector.tensor_mul(qkt[ct][:, ss], pq, cos_sb[:, ss])
                        nc.vector.tensor_add(
                            qkt[ct][:, ss], qkt[ct][:, ss], pp
                        )

            # ---------------- Phase 2: causal attention -------------------
            with (
                tc.tile_pool(name="p2", bufs=1) as p2,
                tc.tile_pool(name="p2ps", bufs=1, space="PSUM") as p2ps,
            ):
                for h in range(GH):
                    ct = h // 2
                    po = (h % 2) * 64
                    qt, kt = qkt[ct], qkt[2 + ct]
                    for sp in range(NSPAN):
                        qe = 512 * (sp + 1)
                        pvps = p2ps.tile(
                            [65, 512], F32, tag="pspv", bufs=2, name="pspv"
                        )
                        njt = 4 * (sp + 1)
                        for j in range(njt):
                            q0 = max(512 * sp, 128 * j)
                            w = qe - q0
                            sps = p2ps.tile(
                                [128, 512], F32, tag="pss", bufs=2, name="pss"
                            )
                            nc.tensor.matmul(
                                sps[:, :w],
                                kt[po : po + 64, 128 * j : 128 * (j + 1)],
                                qt[po : po + 64, q0:qe],
                                start=True,
                                stop=True,
                            )
                            et = p2.tile([128, 512], F32R, tag="et", bufs=3, name="et")
                            nc.scalar.activation(
                                out=et[:, :w],
                                in_=sps[:, :w],
                                func=mybir.ActivationFunctionType.Exp,
                                scale=0.125,
                            )
                            if j >= 4 * sp:
                                # diagonal block: zero out k > q
                                nc.gpsimd.tensor_mul(
                                    et[:, :128], et[:, :128], tri_sb
                                )
                            nc.tensor.matmul(
                                pvps[:, q0 - 512 * sp :],
                                vaug[j][:, 65 * h : 65 * (h + 1)],
                                et[:, :w],
                                start=(j == 0),
                                stop=(j == njt - 1),
                            )
                        # normalize: attn = pv[0:64] * (1 / colsum) broadcast
                        r = p2.tile([65, 512], F32R, tag="r", bufs=2, name="r")
                        with nc.allow_low_precision(
                            reason="fp32r softmax denominator (~1e-3 relerr ok)"
                        ):
                            nc.vector.reciprocal(
                                out=r[64:65, :], in_=pvps[64:65, :]
                            )
                        rb = p2ps.tile([64, 512], F32, tag="psrb", bufs=2, name="psrb")
                        nc.tensor.matmul(
                            rb, ones[64:65, 0:64], r[64:65, :], start=True, stop=True
                        )
                        rbs = p2.tile([64, 512], F32, tag="rbs", bufs=2, name="rbs")
                        nc.scalar.copy(rbs, rb)
                        nc.vector.tensor_mul(
                            attn[ct][po : po + 64, 512 * sp : qe],
                            pvps[0:64, :],
                            rbs,
                        )

            # ---------------- Phase 3: output projection ------------------
            with (
                tc.tile_pool(name="p3", bufs=1) as p3,
                tc.tile_pool(name="p3ps", bufs=1, space="PSUM") as p3ps,
            ):
                wproj_sb = []
                for p in range(2):
                    t = p3.tile([128, C], F32R, tag="wproj", bufs=2, name=f"wproj{p}")
                    nc.sync.dma_start(out=t, in_=wproj_d[p, :, :])
                    wproj_sb.append(t)
                for it in range(NT):
                    ts = slice(128 * it, 128 * (it + 1))
                    pp = p3ps.tile([128, C], F32, tag="psproj", bufs=2, name="psproj")
                    for nh in range(2):
                        ns = slice(512 * nh, 512 * (nh + 1))
                        for p in range(2):
                            nc.tensor.matmul(
                                pp[:, ns],
                                attn[p][:, ts],
                                wproj_sb[p][:, ns],
                                start=(p == 0),
                                stop=(p == 1),
                            )
                    ob = p3.tile([128, C], F32, tag="ob", bufs=3, name="ob")
                    if it % 2 == 0:
                        nc.scalar.copy(ob, pp)
                    else:
                        nc.vector.tensor_copy(ob, pp)
                    nc.sync.dma_start(out=out_d[ts, :], in_=ob)

    nc.compile()
    return nc


_NC = None


def _get_nc():
    global _NC
    if _NC is None:
        _NC = _build()
    return _NC


def _rope_tables():
    theta = (10000.0 ** (-np.arange(0, DH, 2, dtype=np.float32) / DH)).astype(
        np.float32
    )
    t = np.arange(T, dtype=np.float32)
    sinusoid = np.outer(t, theta).astype(np.float32)  # [T, DH/2]
    sin = np.concatenate([np.sin(sinusoid), np.sin(sinusoid)], axis=1)  # [T, DH]
    cos = np.concatenate([np.cos(sinusoid), np.cos(sinusoid)], axis=1)
    cosT = cos.T  # [DH, T]
    sinT = sin.T
    # sin_perm[e] = sin[(e+32) % 64]
    idx = (np.arange(DH) + 32) % DH
    sinTp = sinT[idx]
    cos2 = np.ascontiguousarray(np.concatenate([cosT, cosT], axis=0))  # [128, T]
    sinp2 = np.ascontiguousarray(np.concatenate([sinTp, sinTp], axis=0))
    return cos2, sinp2


def _perm_matrix():
    p = np.zeros((128, 128), dtype=np.float32)
    for m in range(128):
        blk = m // 64
        k = blk * 64 + (m % 64 + 32) % 64
        p[k, m] = 1.0
    return p


def _tri_matrix():
    # tri[k, q] = 1 if k <= q (keep), else 0  (causal in S^T layout)
    return np.triu(np.ones((128, 128), dtype=np.float32))


def _prepare_in_maps(x, w_qkv, b_qkv, w_proj):
    x = np.asarray(x, dtype=np.float32)
    w_qkv = np.asarray(w_qkv, dtype=np.float32)
    b_qkv = np.asarray(b_qkv, dtype=np.float32)
    w_proj = np.asarray(w_proj, dtype=np.float32)

    cos2, sinp2 = _rope_tables()
    perm = _perm_matrix()
    tri = _tri_matrix()
    xTs = [np.ascontiguousarray(x[b].T) for b in range(B)]

    in_maps = []
    for c in range(N_CORES):
        b, g = divmod(c, 4)
        h0 = g * GH  # first head of the group
        qcols = w_qkv[:, h0 * DH : (h0 + GH) * DH]
        kcols = w_qkv[:, C + h0 * DH : C + (h0 + GH) * DH]
        wqk = np.ascontiguousarray(np.concatenate([qcols, kcols], axis=1))
        wv = np.zeros((C, VA), dtype=np.float32)
        bv = np.zeros((1, VA), dtype=np.float32)
        for j in range(GH):
            src = 2 * C + (h0 + j) * DH
            wv[:, j * 65 : j * 65 + DH] = w_qkv[:, src : src + DH]
            bv[0, j * 65 : j * 65 + DH] = b_qkv[src : src + DH]
            bv[0, j * 65 + DH] = 1.0
        bqk = np.concatenate(
            [b_qkv[h0 * DH : (h0 + GH) * DH], b_qkv[C + h0 * DH : C + (h0 + GH) * DH]]
        ).reshape(1, QK_COLS)
        wproj = np.stack(
            [
                w_proj[(h0 + 2 * p) * DH : (h0 + 2 * p + 2) * DH, :]
                for p in range(2)
            ]
        )
        in_maps.append(
            {
                "xT": xTs[b],
                "wqk": wqk,
                "wv": np.ascontiguousarray(wv),
                "bqk": np.ascontiguousarray(bqk),
                "bv": bv,
                "cosT": cos2,
                "sinTp": sinp2,
                "perm": perm,
                "tri": tri,
                "wproj": np.ascontiguousarray(wproj),
            }
        )
    return in_maps


def run(x, w_qkv, b_qkv, w_proj, b_proj, trace=False, tmpdir=None):
    nc = _get_nc()
    in_maps = _prepare_in_maps(x, w_qkv, b_qkv, w_proj)
    res = run_bass_kernel_spmd(
        nc, in_maps, list(range(N_CORES)), trace=trace, tmpdir=tmpdir
    )
    b_proj = np.asarray(b_proj, dtype=np.float32)
    out = np.empty((B, T, C), dtype=np.float32)
    for b in range(B):
        acc = res.results[4 * b]["out"].astype(np.float32)
        for g in range(1, 4):
            acc = acc + res.results[4 * b + g]["out"]
        out[b] = acc + b_proj
    return out, res


def kernel(x, w_qkv, b_qkv, w_proj, b_proj):
    out, _ = run(x, w_qkv, b_qkv, w_proj, b_proj, trace=False)
    return out


# revision 13
# speedup vs baseline: 1.2130x; 1.2130x over previous
"""Multi-head causal attention with RoPE on 8 Trainium2 NeuronCores.

Reference computation (B=2, T=2048, C=1024, H=16, Dh=64, fp32):
    qkv = x @ w_qkv + b_qkv ; split q,k,v ; RoPE(q), RoPE(k)
    attn = softmax_causal(q k^T / sqrt(Dh)) @ v ; out = attn @ w_proj + b_proj

Sharding: core c = b*4 + g handles batch b and head group g (heads 4g..4g+3).
Data-parallel over batch, tensor-parallel over heads (w_qkv column-split,
w_proj row-split).  Each core emits a partial [T, C] projection output; the
host sums the 4 per-batch partials and adds b_proj.

Per-core kernel.  All heavy matmuls run in bf16 (1 PE cycle/row, and low
enough power to stay out of the HAM clock throttle that fp32r's multi-pass
mode triggers); accumulation is always fp32 in PSUM, end-to-end rel err
~5e-3 of output absmax.
  - x^T is pre-transposed on the host, so QKV runs with weights stationary
    producing Q^T/K^T directly in [head_dim, T] layout; V in [T, head_dim].
  - biases are folded into the PSUM accumulation as rank-1 (K=1) matmuls.
  - RoPE: rope(q) = q*cos + shift32(q*sin_perm), the partition shift done
    with a constant 128x128 0/1 permutation matmul.
  - attention per head: S^T tile = K_j Q^T (scores transposed, so the
    softmax sum direction matches the PE contraction), exp on ACT with the
    1/sqrt(Dh) scale fused (no max subtraction: scores are ~N(0,1); fp32
    exp cannot overflow), causal via narrowing each k-tile's q-range plus
    one triangular mask multiply on the diagonal 128x128 block.
  - V is augmented with a ones column so the PV matmul also emits the
    softmax denominator; 1/denom computed on ACT as exp(-ln(d)) (DVE
    reciprocal is 8 cycles/elem, ACT splines are 2x1), then broadcast
    across partitions with a stride-0 DMA.
  - projection: per head-pair stationary attn^T tiles vs w_proj rows.
"""

import numpy as np
import ml_dtypes

import concourse.bacc as bacc
import concourse.bass as bass
import concourse.mybir as mybir
from concourse.tile import TileContext
from concourse.bass_utils import run_bass_kernel_spmd

F32 = mybir.dt.float32
BF16 = mybir.dt.bfloat16
NPBF16 = np.dtype(ml_dtypes.bfloat16)

B, T, C = 2, 2048, 1024
H, DH = 16, 64
GH = 4  # heads per core
N_CORES = 8
NCHUNK = C // 128  # 8 contraction chunks
NT = T // 128  # 16 token tiles
NSPAN = T // 512  # 4 query spans
QK_COLS = 2 * GH * DH  # 512 = q cols (256) + k cols (256)
VA = GH * (DH + 1)  # 260 = v cols augmented with ones column per head
EXP = mybir.ActivationFunctionType.Exp
LOG = mybir.ActivationFunctionType.Ln


def _build():
    nc = bacc.Bacc("TRN2", target_bir_lowering=False, debug=False, num_devices=N_CORES)

    xT = nc.dram_tensor("xT", [C, T], BF16, kind="ExternalInput")
    wqk = nc.dram_tensor("wqk", [C, QK_COLS], BF16, kind="ExternalInput")
    wv = nc.dram_tensor("wv", [C, VA], BF16, kind="ExternalInput")
    bqk_d = nc.dram_tensor("bqk", [1, QK_COLS], BF16, kind="ExternalInput")
    bv_d = nc.dram_tensor("bv", [1, VA], BF16, kind="ExternalInput")
    cos_d = nc.dram_tensor("cosT", [128, T], F32, kind="ExternalInput")
    sinp_d = nc.dram_tensor("sinTp", [128, T], F32, kind="ExternalInput")
    perm_d = nc.dram_tensor("perm", [128, 128], BF16, kind="ExternalInput")
    tri_d = nc.dram_tensor("tri", [128, 128], BF16, kind="ExternalInput")
    wproj_d = nc.dram_tensor("wproj", [2, 128, C], BF16, kind="ExternalInput")
    out_d = nc.dram_tensor("out", [T, C], F32, kind="ExternalOutput")

    with TileContext(nc) as tc:
        with tc.tile_pool(name="persist", bufs=1) as pers:
            ones = pers.tile([1, 512], BF16, tag="ones")
            nc.vector.memset(ones, 1.0)
            ones_ff = pers.tile([128, 64], F32, tag="ones_ff")
            nc.vector.memset(ones_ff, 1.0)
            ones_r = pers.tile([128, 64], mybir.dt.float32r, tag="ones_r")
            nc.vector.tensor_copy(ones_r, ones_ff)
            cos_sb = pers.tile([128, T], F32, tag="cos")
            nc.sync.dma_start(out=cos_sb, in_=cos_d[:, :])
            sinp_sb = pers.tile([128, T], F32, tag="sinp")
            nc.sync.dma_start(out=sinp_sb, in_=sinp_d[:, :])
            perm_sb = pers.tile([128, 128], BF16, tag="perm")
            nc.sync.dma_start(out=perm_sb, in_=perm_d[:, :])
            tri_sb = pers.tile([128, 128], BF16, tag="tri")
            nc.sync.dma_start(out=tri_sb, in_=tri_d[:, :])
            bqk_sb = pers.tile([1, QK_COLS], BF16, tag="bqk")
            nc.sync.dma_start(out=bqk_sb, in_=bqk_d[:, :])
            bv_sb = pers.tile([1, VA], BF16, tag="bv")
            nc.sync.dma_start(out=bv_sb, in_=bv_d[:, :])

            # Outputs of phase 1 (live into phase 2/3)
            qkt = []  # 4 tiles [128, T]: Q heads(0,1), Q(2,3), K(0,1), K(2,3)
            for i in range(4):
                t = pers.tile([128, T], BF16, tag="qkt", bufs=4, name=f"qkt{i}")
                qkt.append(t)
            vaug = []  # 16 tiles [128, VA], k-tile-major natural layout V
            for j in range(NT):
                t = pers.tile([128, VA], BF16, tag="vaug", bufs=NT, name=f"vaug{j}")
                vaug.append(t)
            attn = []  # 2 tiles [128, T]: normalized attn^T for head pairs
            for p in range(2):
                t = pers.tile([128, T], BF16, tag="attn", bufs=2, name=f"attn{p}")
                attn.append(t)

            # ---------------- Phase 1: QKV projection + RoPE ----------------
            with (
                tc.tile_pool(name="p1", bufs=1) as p1,
                tc.tile_pool(name="p1ps", bufs=1, space="PSUM") as p1ps,
            ):
                xt = []
                for kc in range(NCHUNK):
                    t = p1.tile([128, T], BF16, tag="xt", bufs=NCHUNK, name=f"xt{kc}")
                    nc.sync.dma_start(out=t, in_=xT[128 * kc : 128 * (kc + 1), :])
                    xt.append(t)
                wqk_t = []
                for kc in range(NCHUNK):
                    t = p1.tile(
                        [128, QK_COLS], BF16, tag="wqk", bufs=NCHUNK, name=f"wqk{kc}"
                    )
                    nc.sync.dma_start(out=t, in_=wqk[128 * kc : 128 * (kc + 1), :])
                    wqk_t.append(t)
                wv_t = []
                for kc in range(NCHUNK):
                    t = p1.tile([128, VA], BF16, tag="wv", bufs=NCHUNK, name=f"wv{kc}")
                    nc.sync.dma_start(out=t, in_=wv[128 * kc : 128 * (kc + 1), :])
                    wv_t.append(t)

                # V natural layout: for each token tile, [128 tok, VA cols]
                for it in range(NT):
                    pv = p1ps.tile([128, VA], F32, tag="psv", bufs=2, name="psv")
                    ts = slice(128 * it, 128 * (it + 1))
                    for kc in range(NCHUNK):
                        nc.tensor.matmul(
                            pv, xt[kc][:, ts], wv_t[kc], start=(kc == 0), stop=False
                        )
                    # bias (includes the ones column): pv[t, c] += bv[c]
                    nc.tensor.matmul(
                        pv, ones[0:1, 0:128], bv_sb, start=False, stop=True
                    )
                    nc.vector.tensor_copy(vaug[it], pv)

                # Q^T / K^T col-tiles with fused bias + RoPE
                # (emit K first so attention's S^T matmuls unblock earliest)
                for ct in (2, 3, 0, 1):
                    cs = slice(128 * ct, 128 * (ct + 1))
                    for sp in range(NSPAN):
                        ss = slice(512 * sp, 512 * (sp + 1))
                        pq = p1ps.tile([128, 512], F32, tag="psqk", bufs=2, name="psqk")
                        for kc in range(NCHUNK):
                            nc.tensor.matmul(
                                pq,
                                wqk_t[kc][:, cs],
                                xt[kc][:, ss],
                                start=(kc == 0),
                                stop=False,
                            )
                        nc.tensor.matmul(
                            pq, bqk_sb[0:1, cs], ones, start=False, stop=True
                        )
                        # rope: qkt = pq*cos + perm @ (pq*sin_perm)
                        t2 = p1.tile([128, 512], BF16, tag="t2", bufs=3, name="t2")
                        nc.vector.tensor_mul(t2, pq, sinp_sb[:, ss])
                        pp = p1ps.tile(
                            [128, 512], F32, tag="psperm", bufs=2, name="psperm"
                        )
                        nc.tensor.matmul(pp, perm_sb, t2, start=True, stop=True)
                        nc.vector.tensor_mul(qkt[ct][:, ss], pq, cos_sb[:, ss])
                        nc.vector.tensor_add(qkt[ct][:, ss], qkt[ct][:, ss], pp)

            # ---------------- Phase 2: causal attention -------------------
            with (
                tc.tile_pool(name="p2", bufs=1) as p2,
                tc.tile_pool(name="p2ps", bufs=1, space="PSUM") as p2ps,
            ):
                for h in range(GH):
                    ct = h // 2
                    po = (h % 2) * 64
                    qt, kt = qkt[ct], qkt[2 + ct]
                    pvps = [
                        p2ps.tile([65, 512], F32, tag="pspv", bufs=4, name=f"pspv{s}")
                        for s in range(NSPAN)
                    ]
                    # chunk list: (j, s, q0, w); S^T then exp, PV lags by one
                    # chunk so the PE never stalls on ACT's exp.
                    chunks = []
                    for j in range(NT):
                        for s in range(j // 4, NSPAN):
                            q0 = max(512 * s, 128 * j)
                            chunks.append((j, s, q0, 512 * (s + 1) - q0))
                    pending = None
                    for ci, (j, s, q0, w) in enumerate(chunks):
                        sps = p2ps.tile([128, 512], F32, tag="pss", bufs=3, name="pss")
                        nc.tensor.matmul(
                            sps[:, :w],
                            kt[po : po + 64, 128 * j : 128 * (j + 1)],
                            qt[po : po + 64, q0 : q0 + w],
                            start=True,
                            stop=True,
                        )
                        et = p2.tile([128, 512], BF16, tag="et", bufs=6, name="et")
                        nc.scalar.activation(
                            out=et[:, :w], in_=sps[:, :w], func=EXP, scale=0.125
                        )
                        if s == j // 4:
                            # diagonal block: zero out k > q
                            nc.gpsimd.tensor_mul(et[:, :128], et[:, :128], tri_sb)
                        if pending is not None:
                            pj, ps_, pq0, pw, pet = pending
                            nc.tensor.matmul(
                                pvps[ps_][:, pq0 - 512 * ps_ :],
                                vaug[pj][:, 65 * h : 65 * (h + 1)],
                                pet[:, :pw],
                                start=(pj == 0),
                                stop=(pj == 4 * ps_ + 3),
                            )
                        pending = (j, s, q0, w, et)
                    pj, ps_, pq0, pw, pet = pending
                    nc.tensor.matmul(
                        pvps[ps_][:, pq0 - 512 * ps_ :],
                        vaug[pj][:, 65 * h : 65 * (h + 1)],
                        pet[:, :pw],
                        start=(pj == 0),
                        stop=True,
                    )
                    # normalize: attn = pv[0:64] * (1/colsum), with
                    # 1/colsum = exp(-ln(colsum)) on ACT, broadcast across
                    # partitions by a stride-0 DMA.
                    for s in range(NSPAN):
                        rln = p2.tile([65, 512], F32, tag="rln", bufs=2, name="rln")
                        nc.scalar.activation(
                            out=rln[64:65, :], in_=pvps[s][64:65, :], func=LOG
                        )
                        r = p2.tile(
                            [65, 512], mybir.dt.float32r, tag="r", bufs=2, name="r"
                        )
                        nc.scalar.activation(
                            out=r[64:65, :], in_=rln[64:65, :], func=EXP, scale=-1.0
                        )
                        rb = p2ps.tile([64, 512], F32, tag="psrb", bufs=1, name="psrb")
                        nc.tensor.matmul(
                            rb,
                            ones_r[64:65, :],
                            r[64:65, :],
                            start=True,
                            stop=True,
                        )
                        rbs = p2.tile([64, 512], F32, tag="rbs", bufs=2, name="rbs")
                        nc.vector.tensor_copy(rbs, rb)
                        nc.vector.tensor_mul(
                            attn[ct][po : po + 64, 512 * s : 512 * (s + 1)],
                            pvps[s][0:64, :],
                            rbs,
                        )

            # ---------------- Phase 3: output projection ------------------
            with (
                tc.tile_pool(name="p3", bufs=1) as p3,
                tc.tile_pool(name="p3ps", bufs=1, space="PSUM") as p3ps,
            ):
                wproj_sb = []
                for p in range(2):
                    t = p3.tile([128, C], BF16, tag="wproj", bufs=2, name=f"wproj{p}")
                    nc.sync.dma_start(out=t, in_=wproj_d[p, :, :])
                    wproj_sb.append(t)
                for it in range(NT):
                    ts = slice(128 * it, 128 * (it + 1))
                    pp = p3ps.tile([128, C], F32, tag="psproj", bufs=2, name="psproj")
                    for p in range(2):
                        for nh in range(2):
                            ns = slice(512 * nh, 512 * (nh + 1))
                            nc.tensor.matmul(
                                pp[:, ns],
                                attn[p][:, ts],
                                wproj_sb[p][:, ns],
                                start=(p == 0),
                                stop=(p == 1),
                            )
                    ob = p3.tile([128, C], F32, tag="ob", bufs=4, name="ob")
                    if it % 2 == 0:
                        nc.scalar.copy(ob, pp)
                    else:
                        nc.vector.tensor_copy(ob, pp)
                    nc.sync.dma_start(out=out_d[ts, :], in_=ob)

    nc.compile()
    return nc


_NC = None


def _get_nc():
    global _NC
    if _NC is None:
        _NC = _build()
    return _NC


def _rope_tables():
    theta = (10000.0 ** (-np.arange(0, DH, 2, dtype=np.float32) / DH)).astype(
        np.float32
    )
    t = np.arange(T, dtype=np.float32)
    sinusoid = np.outer(t, theta).astype(np.float32)  # [T, DH/2]
    sin = np.concatenate([np.sin(sinusoid), np.sin(sinusoid)], axis=1)  # [T, DH]
    cos = np.concatenate([np.cos(sinusoid), np.cos(sinusoid)], axis=1)
    cosT = cos.T  # [DH, T]
    sinT = sin.T
    # sin_perm[e] = sin[(e+32) % 64]
    idx = (np.arange(DH) + 32) % DH
    sinTp = sinT[idx]
    cos2 = np.ascontiguousarray(np.concatenate([cosT, cosT], axis=0))  # [128, T]
    sinp2 = np.ascontiguousarray(np.concatenate([sinTp, sinTp], axis=0))
    return cos2, sinp2


def _perm_matrix():
    p = np.zeros((128, 128), dtype=np.float32)
    for m in range(128):
        blk = m // 64
        k = blk * 64 + (m % 64 + 32) % 64
        p[k, m] = 1.0
    return p


def _tri_matrix():
    # tri[k, q] = 1 if k <= q (keep), else 0  (causal in S^T layout)
    return np.triu(np.ones((128, 128), dtype=np.float32))


def _bf(a):
    return np.ascontiguousarray(np.asarray(a, dtype=np.float32).astype(NPBF16))


def _prepare_in_maps(x, w_qkv, b_qkv, w_proj):
    x = np.asarray(x, dtype=np.float32)
    w_qkv = np.asarray(w_qkv, dtype=np.float32)
    b_qkv = np.asarray(b_qkv, dtype=np.float32)
    w_proj = np.asarray(w_proj, dtype=np.float32)

    cos2, sinp2 = _rope_tables()
    perm = _bf(_perm_matrix())
    tri = _bf(_tri_matrix())
    xTs = [_bf(x[b].T) for b in range(B)]

    in_maps = []
    for c in range(N_CORES):
        b, g = divmod(c, 4)
        h0 = g * GH  # first head of the group
        qcols = w_qkv[:, h0 * DH : (h0 + GH) * DH]
        kcols = w_qkv[:, C + h0 * DH : C + (h0 + GH) * DH]
        wqk = _bf(np.concatenate([qcols, kcols], axis=1))
        wv = np.zeros((C, VA), dtype=np.float32)
        bv = np.zeros((1, VA), dtype=np.float32)
        for j in range(GH):
            src = 2 * C + (h0 + j) * DH
            wv[:, j * 65 : j * 65 + DH] = w_qkv[:, src : src + DH]
            bv[0, j * 65 : j * 65 + DH] = b_qkv[src : src + DH]
            bv[0, j * 65 + DH] = 1.0
        bqk = np.concatenate(
            [b_qkv[h0 * DH : (h0 + GH) * DH], b_qkv[C + h0 * DH : C + (h0 + GH) * DH]]
        ).reshape(1, QK_COLS)
        wproj = np.stack(
            [w_proj[(h0 + 2 * p) * DH : (h0 + 2 * p + 2) * DH, :] for p in range(2)]
        )
        in_maps.append(
            {
                "xT": xTs[b],
                "wqk": wqk,
                "wv": _bf(wv),
                "bqk": _bf(bqk),
                "bv": _bf(bv),
                "cosT": cos2,
                "sinTp": sinp2,
                "perm": perm,
                "tri": tri,
                "wproj": _bf(wproj),
            }
        )
    return in_maps


def run(x, w_qkv, b_qkv, w_proj, b_proj, trace=False, tmpdir=None):
    nc = _get_nc()
    in_maps = _prepare_in_maps(x, w_qkv, b_qkv, w_proj)
    res = run_bass_kernel_spmd(
        nc, in_maps, list(range(N_CORES)), trace=trace, tmpdir=tmpdir
    )
    b_proj = np.asarray(b_proj, dtype=np.float32)
    out = np.empty((B, T, C), dtype=np.float32)
    for b in range(B):
        acc = res.results[4 * b]["out"].astype(np.float32)
        for g in range(1, 4):
            acc = acc + res.results[4 * b + g]["out"]
        out[b] = acc + b_proj
    return out, res


def kernel(x, w_qkv, b_qkv, w_proj, b_proj):
    out, _ = run(x, w_qkv, b_qkv, w_proj, b_proj, trace=False)
    return out
